# revision 5
# baseline (speedup 1.0000x reference)
"""Trainium2 Bass kernel for nn_DarcyLoss (data-parallel over batch on 8 cores).

loss = mean((model_output - target)^2)
     + mean_b( 0.5 * (sigma_t/0.01) * mean_hw(F_b^2) )
where F = dx(K * dx p) + dy(K * dy p) + f   (2nd-order finite differences,
K = x0_hat[:,0], p = x0_hat[:,1], f = Darcy source term).

Per-core plan (4 images each):
 - Work with the scaled stencil G' = 2*G (integer coefficients, exact in f32).
 - y-derivatives (partition axis): PE matmuls against constant G'^T blocks.
 - x-derivatives (free axis): DVE shifted-AP subtracts + 2-op edge fixups.
 - A (x-part) added into the F' PSUM accumulation via identity matmul.
 - F^2 = Square(0.25*F' + f) fused on ScalarE with accum_out row-sums.
 - MSE: DVE subtract (in-place), ScalarE Square with accum_out.
 - Partition reduction: single ones-matmul -> [1,32] partials -> host f64.
"""

import sys
from contextlib import ExitStack

import numpy as np

sys.path.insert(0, "/opt/trn_rl_repo")

import concourse.bass as bass  # noqa: E402
import concourse.tile as tile  # noqa: E402
from concourse import bacc, mybir  # noqa: E402
from concourse import bass_utils  # noqa: E402

N_CORES = 8
B, H, W = 32, 512, 512
BPC = B // N_CORES  # images per core
F32 = mybir.dt.float32

_SUB = mybir.AluOpType.subtract
_ADD = mybir.AluOpType.add
_MUL = mybir.AluOpType.mult
_SQ = mybir.ActivationFunctionType.Square


def grad_matrix_2x(n: int) -> np.ndarray:
    """G' = 2 * (torch.gradient, spacing=1, edge_order=2) as a dense matrix."""
    G = np.zeros((n, n), np.float32)
    for h in range(1, n - 1):
        G[h, h + 1] = 1.0
        G[h, h - 1] = -1.0
    G[0, 0], G[0, 1], G[0, 2] = -3.0, 4.0, -1.0
    G[n - 1, n - 1], G[n - 1, n - 2], G[n - 1, n - 3] = 3.0, -4.0, 1.0
    return G


def _stencil_free_axis(nc, dst, src):
    """Apply G' along the last (free) axis: dst/src are [128, 4, 512] APs."""
    # interior: dst[..., j] = src[..., j+1] - src[..., j-1]
    nc.vector.tensor_tensor(dst[:, :, 1:511], src[:, :, 2:512], src[:, :, 0:510], _SUB)
    # left edge: -3*s0 + 4*s1 - s2  (two fused scalar_tensor_tensor ops)
    nc.vector.scalar_tensor_tensor(
        dst[:, :, 0:1], src[:, :, 0:1], 3.0, src[:, :, 2:3], _MUL, _ADD
    )
    nc.vector.scalar_tensor_tensor(
        dst[:, :, 0:1], src[:, :, 1:2], 4.0, dst[:, :, 0:1], _MUL, _SUB
    )
    # right edge: 3*s511 - 4*s510 + s509
    nc.vector.scalar_tensor_tensor(
        dst[:, :, 511:512], src[:, :, 510:511], 4.0, src[:, :, 509:510], _MUL, _SUB
    )
    nc.vector.scalar_tensor_tensor(
        dst[:, :, 511:512], src[:, :, 511:512], 3.0, dst[:, :, 511:512], _MUL, _SUB
    )


def _kernel_body(ctx, tc, x0, mo, tg, gt, ident, fbias, out):
    nc = tc.nc

    consts = ctx.enter_context(tc.tile_pool(name="consts", bufs=1))
    gt_sb = consts.tile([128, 2048], F32)
    nc.sync.dma_start(
        gt_sb[:].rearrange("p (jb c) -> p jb c", jb=4),
        gt.rearrange("(jb p) c -> p jb c", p=128),
    )
    id_sb = consts.tile([128, 128], F32)
    nc.sync.dma_start(id_sb[:], ident)
    fb_sb = consts.tile([128, 2], F32)
    nc.sync.dma_start(fb_sb[:], fbias)
    ones_sb = consts.tile([128, 1], F32)
    nc.vector.memset(ones_sb[:], 1.0)
    acc_sb = consts.tile([128, 32], F32)

    x0_pool = ctx.enter_context(tc.tile_pool(name="x0", bufs=2))
    t1x_pool = ctx.enter_context(tc.tile_pool(name="t1x", bufs=2))
    t2x_pool = ctx.enter_context(tc.tile_pool(name="t2x", bufs=2))
    a_pool = ctx.enter_context(tc.tile_pool(name="abuf", bufs=2))
    t2y_pool = ctx.enter_context(tc.tile_pool(name="t2y", bufs=2))
    sq_pool = ctx.enter_context(tc.tile_pool(name="sq", bufs=3))
    mo_pool = ctx.enter_context(tc.tile_pool(name="mo", bufs=2))
    tg_pool = ctx.enter_context(tc.tile_pool(name="tg", bufs=2))
    p_t1y = ctx.enter_context(tc.tile_pool(name="pt1y", bufs=2, space="PSUM"))
    p_fp = ctx.enter_context(tc.tile_pool(name="pfp", bufs=2, space="PSUM"))
    p_red = ctx.enter_context(tc.tile_pool(name="pred", bufs=1, space="PSUM"))

    mo_r = mo.rearrange("b c (n p) w -> (b c) p n w", p=128)
    tg_r = tg.rearrange("b c (n p) w -> (b c) p n w", p=128)

    def gtb(j, i):
        """lhsT block: G'^T[j*128:(j+1)*128, i*128:(i+1)*128]."""
        return gt_sb[:, j * 512 + i * 128 : j * 512 + (i + 1) * 128]

    for img in range(BPC):
        x0t = x0_pool.tile([128, 4096], F32)
        nc.sync.dma_start(
            x0t[:].rearrange("p (c n w) -> p c n w", c=2, n=4),
            x0[img].rearrange("c (n p) w -> p c n w", p=128),
        )
        Kf = x0t[:, 0:2048]
        Pf = x0t[:, 2048:4096]
        Pv = Pf.rearrange("p (n w) -> p n w", n=4)

        # ---- x-branch (free axis) on DVE ----
        t1x = t1x_pool.tile([128, 2048], F32)
        _stencil_free_axis(nc, t1x[:].rearrange("p (n w) -> p n w", n=4), Pv)
        t2x = t2x_pool.tile([128, 2048], F32)
        nc.vector.tensor_tensor(t2x[:], Kf, t1x[:], _MUL)
        a_t = a_pool.tile([128, 2048], F32)
        _stencil_free_axis(
            nc,
            a_t[:].rearrange("p (n w) -> p n w", n=4),
            t2x[:].rearrange("p (n w) -> p n w", n=4),
        )

        # ---- y-branch (partition axis) on PE ----
        t2y = t2y_pool.tile([128, 2048], F32)
        for i in range(4):
            js = [j for j in (i - 1, i, i + 1) if 0 <= j <= 3]
            t1y = p_t1y.tile([128, 512], F32)
            for k, j in enumerate(js):
                nc.tensor.matmul(
                    t1y[:],
                    gtb(j, i),
                    Pf[:, j * 512 : (j + 1) * 512],
                    start=(k == 0),
                    stop=(k == len(js) - 1),
                )
            nc.vector.tensor_tensor(
                t2y[:, i * 512 : (i + 1) * 512],
                Kf[:, i * 512 : (i + 1) * 512],
                t1y[:],
                _MUL,
            )

        # ---- F' accumulation + fused square/row-sum ----
        c0 = img * 6
        for i in range(4):
            js = [j for j in (i - 1, i, i + 1) if 0 <= j <= 3]
            fp = p_fp.tile([128, 512], F32)
            for k, j in enumerate(js):
                nc.tensor.matmul(
                    fp[:],
                    gtb(j, i),
                    t2y[:, j * 512 : (j + 1) * 512],
                    start=(k == 0),
                    stop=False,
                )
            nc.tensor.matmul(
                fp[:], id_sb[:], a_t[:, i * 512 : (i + 1) * 512], start=False, stop=True
            )
            sq = sq_pool.tile([128, 512], F32)
            if i == 0:
                nc.scalar.activation(
                    sq[:, 0:64], fp[:, 0:64], _SQ,
                    bias=fb_sb[:, 0:1], scale=0.25,
                    accum_out=acc_sb[:, c0 : c0 + 1],
                )
                nc.scalar.activation(
                    sq[:, 64:512], fp[:, 64:512], _SQ, scale=0.25,
                    accum_out=acc_sb[:, c0 + 1 : c0 + 2],
                )
            elif i == 3:
                nc.scalar.activation(
                    sq[:, 0:448], fp[:, 0:448], _SQ, scale=0.25,
                    accum_out=acc_sb[:, c0 + 4 : c0 + 5],
                )
                nc.scalar.activation(
                    sq[:, 448:512], fp[:, 448:512], _SQ,
                    bias=fb_sb[:, 1:2], scale=0.25,
                    accum_out=acc_sb[:, c0 + 5 : c0 + 6],
                )
            else:
                nc.scalar.activation(
                    sq[:], fp[:], _SQ, scale=0.25,
                    accum_out=acc_sb[:, c0 + 1 + i : c0 + 2 + i],
                )

        # ---- MSE tiles (2 per image) ----
        for t in (2 * img, 2 * img + 1):
            mot = mo_pool.tile([128, 2048], F32)
            nc.sync.dma_start(mot[:].rearrange("p (n w) -> p n w", n=4), mo_r[t])
            tgt_ = tg_pool.tile([128, 2048], F32)
            nc.sync.dma_start(tgt_[:].rearrange("p (n w) -> p n w", n=4), tg_r[t])
            nc.vector.tensor_tensor(mot[:], mot[:], tgt_[:], _SUB)
            nc.scalar.activation(
                mot[:], mot[:], _SQ, accum_out=acc_sb[:, 24 + t : 25 + t]
            )

    # ---- final partition reduction: [128,32] -> [1,32] ----
    red = p_red.tile([1, 32], F32)
    nc.tensor.matmul(red[:], ones_sb[:], acc_sb[:])
    out_sb = consts.tile([1, 32], F32)
    nc.scalar.copy(out_sb[:], red[:])
    nc.sync.dma_start(out, out_sb[:])


_NC_CACHE = {}


def build_program():
    if "nc" in _NC_CACHE:
        return _NC_CACHE["nc"]
    nc = bacc.Bacc(
        "TRN2", target_bir_lowering=False, debug=False, num_devices=N_CORES
    )
    x0 = nc.dram_tensor("x0", [BPC, 2, H, W], F32, kind="ExternalInput").ap()
    mo = nc.dram_tensor("mo", [BPC, 2, H, W], F32, kind="ExternalInput").ap()
    tg = nc.dram_tensor("tg", [BPC, 2, H, W], F32, kind="ExternalInput").ap()
    gt = nc.dram_tensor("gt", [H, H], F32, kind="ExternalInput").ap()
    ident = nc.dram_tensor("ident", [128, 128], F32, kind="ExternalInput").ap()
    fbias = nc.dram_tensor("fbias", [128, 2], F32, kind="ExternalInput").ap()
    out = nc.dram_tensor("partials", [1, 32], F32, kind="ExternalOutput").ap()
    with tile.TileContext(nc) as tc, ExitStack() as ctx:
        _kernel_body(ctx, tc, x0, mo, tg, gt, ident, fbias, out)
    nc.compile()
    _NC_CACHE["nc"] = nc
    return nc


def make_in_maps(model_output, target, x0_hat):
    gt_np = np.ascontiguousarray(grad_matrix_2x(H).T)
    id_np = np.eye(128, dtype=np.float32)
    fb_np = np.zeros((128, 2), np.float32)
    fb_np[0:64, 0] = 10.0  # f source, rows 0:64 of block n=0 (cols 0:64)
    fb_np[64:128, 1] = -10.0  # f source, rows 448:512 of block n=3 (cols 448:512)
    in_maps = []
    for c in range(N_CORES):
        sl = slice(c * BPC, (c + 1) * BPC)
        in_maps.append(
            {
                "x0": np.ascontiguousarray(x0_hat[sl], dtype=np.float32),
                "mo": np.ascontiguousarray(model_output[sl], dtype=np.float32),
                "tg": np.ascontiguousarray(target[sl], dtype=np.float32),
                "gt": gt_np,
                "ident": id_np,
                "fbias": fb_np,
            }
        )
    return in_maps


def combine_partials(partials_per_core, sigma_t):
    """partials: per core [1,32] f32 -> final scalar loss (host f64 math)."""
    total_mse = 0.0
    total_pde = 0.0
    for c in range(N_CORES):
        p = np.asarray(partials_per_core[c]).reshape(32).astype(np.float64)
        total_mse += p[24:32].sum()
        for img in range(BPC):
            s = p[img * 6 : (img + 1) * 6].sum()
            total_pde += 50.0 * float(sigma_t[c * BPC + img]) * s / float(H * W)
    loss = total_mse / float(B * 2 * H * W) + total_pde / float(B)
    return np.float32(loss)


def kernel(model_output, target, x0_hat, sigma_t):
    nc = build_program()
    in_maps = make_in_maps(model_output, target, x0_hat)
    res = bass_utils.run_bass_kernel_spmd(nc, in_maps, core_ids=list(range(N_CORES)))
    partials = [res.results[c]["partials"] for c in range(N_CORES)]
    return combine_partials(partials, np.asarray(sigma_t))


# revision 11
# speedup vs baseline: 1.2240x; 1.2240x over previous
"""Trainium2 Bass kernel for nn_DarcyLoss (data-parallel over batch on 8 cores).

loss = mean((model_output - target)^2)
     + mean_b( 0.5 * (sigma_t/0.01) * mean_hw(F_b^2) )
where F = dx(K * dx p) + dy(K * dy p) + f   (2nd-order finite differences,
K = x0_hat[:,0], p = x0_hat[:,1], f = Darcy source term).

Per-core plan (4 images each):
 - Work with the scaled stencil G' = 2*G (integer coefficients, exact in f32).
 - y-derivatives (partition axis): PE matmuls against constant G'^T blocks.
 - x-derivatives (free axis): DVE shifted-AP subtracts + 2-op edge fixups.
 - A (x-part) added into the F' PSUM accumulation via identity matmul.
 - F^2 = Square(0.25*F' + f) fused on ScalarE with accum_out row-sums.
 - MSE: DVE subtract (in-place), ScalarE Square with accum_out.
 - Partition reduction: single ones-matmul -> [1,32] partials -> host f64.
"""

import sys
from contextlib import ExitStack

import numpy as np

sys.path.insert(0, "/opt/trn_rl_repo")

import concourse.bass as bass  # noqa: E402
import concourse.tile as tile  # noqa: E402
from concourse import bacc, mybir  # noqa: E402
from concourse import bass_utils  # noqa: E402

N_CORES = 8
B, H, W = 32, 512, 512
BPC = B // N_CORES  # images per core
F32 = mybir.dt.float32
F32R = mybir.dt.float32r  # single-pass PE mode: fp32 storage, 11-bit mantissa

_SUB = mybir.AluOpType.subtract
_ADD = mybir.AluOpType.add
_MUL = mybir.AluOpType.mult
_SQ = mybir.ActivationFunctionType.Square


def grad_matrix_2x(n: int) -> np.ndarray:
    """G' = 2 * (torch.gradient, spacing=1, edge_order=2) as a dense matrix."""
    G = np.zeros((n, n), np.float32)
    for h in range(1, n - 1):
        G[h, h + 1] = 1.0
        G[h, h - 1] = -1.0
    G[0, 0], G[0, 1], G[0, 2] = -3.0, 4.0, -1.0
    G[n - 1, n - 1], G[n - 1, n - 2], G[n - 1, n - 3] = 3.0, -4.0, 1.0
    return G


def _stencil_free_axis(nc, dst, src):
    """Apply G' along the last (free) axis: dst/src are [128, 4, 512] APs."""
    # re-reads of dst (edge second ops) must be plain f32 even if dst is f32r
    dre = dst.bitcast(F32) if dst.dtype == F32R else dst
    # interior: dst[..., j] = src[..., j+1] - src[..., j-1]
    nc.vector.tensor_tensor(dst[:, :, 1:511], src[:, :, 2:512], src[:, :, 0:510], _SUB)
    # left edge: -3*s0 + 4*s1 - s2  (two fused scalar_tensor_tensor ops)
    nc.vector.scalar_tensor_tensor(
        dst[:, :, 0:1], src[:, :, 0:1], 3.0, src[:, :, 2:3], _MUL, _ADD
    )
    nc.vector.scalar_tensor_tensor(
        dst[:, :, 0:1], src[:, :, 1:2], 4.0, dre[:, :, 0:1], _MUL, _SUB
    )
    # right edge: 3*s511 - 4*s510 + s509
    nc.vector.scalar_tensor_tensor(
        dst[:, :, 511:512], src[:, :, 510:511], 4.0, src[:, :, 509:510], _MUL, _SUB
    )
    nc.vector.scalar_tensor_tensor(
        dst[:, :, 511:512], src[:, :, 511:512], 3.0, dre[:, :, 511:512], _MUL, _SUB
    )


def _kernel_body(ctx, tc, xk, xp, mo, tg, gt, ident, fbias, out):
    nc = tc.nc

    consts = ctx.enter_context(tc.tile_pool(name="consts", bufs=1))
    gt_sb = consts.tile([128, 2048], F32R)
    nc.sync.dma_start(
        gt_sb[:].rearrange("p (jb c) -> p jb c", jb=4),
        gt.rearrange("(jb p) c -> p jb c", p=128),
    )
    id_sb = consts.tile([128, 128], F32R)
    nc.sync.dma_start(id_sb[:], ident)
    fb_sb = consts.tile([128, 2], F32)
    nc.sync.dma_start(fb_sb[:], fbias)
    ones_sb = consts.tile([128, 1], F32)
    nc.vector.memset(ones_sb[:], 1.0)
    acc_sb = consts.tile([128, 32], F32)

    xk_pool = ctx.enter_context(tc.tile_pool(name="xk", bufs=2))
    xp_pool = ctx.enter_context(tc.tile_pool(name="xp", bufs=2))
    t1x_pool = ctx.enter_context(tc.tile_pool(name="t1x", bufs=2))
    t2x_pool = ctx.enter_context(tc.tile_pool(name="t2x", bufs=2))
    a_pool = ctx.enter_context(tc.tile_pool(name="abuf", bufs=2))
    t2y_pool = ctx.enter_context(tc.tile_pool(name="t2y", bufs=2))
    sq_pool = ctx.enter_context(tc.tile_pool(name="sq", bufs=3))
    mo_pool = ctx.enter_context(tc.tile_pool(name="mo", bufs=2))
    tg_pool = ctx.enter_context(tc.tile_pool(name="tg", bufs=2))
    p_t1y = ctx.enter_context(tc.tile_pool(name="pt1y", bufs=1, space="PSUM"))
    p_fp = ctx.enter_context(tc.tile_pool(name="pfp", bufs=2, space="PSUM"))
    p_red = ctx.enter_context(tc.tile_pool(name="pred", bufs=1, space="PSUM"))

    mo_r = mo.rearrange("b c (n p) w -> (b c) p n w", p=128)
    tg_r = tg.rearrange("b c (n p) w -> (b c) p n w", p=128)

    def gtb(j, i):
        """lhsT block: G'^T[j*128:(j+1)*128, i*128:(i+1)*128]."""
        return gt_sb[:, j * 512 + i * 128 : j * 512 + (i + 1) * 128]

    for img in range(BPC):
        kt = xk_pool.tile([128, 2048], F32)
        nc.sync.dma_start(
            kt[:].rearrange("p (n w) -> p n w", n=4),
            xk[img].rearrange("(n p) w -> p n w", p=128),
        )
        pt = xp_pool.tile([128, 2048], F32R)
        nc.sync.dma_start(
            pt[:].rearrange("p (n w) -> p n w", n=4),
            xp[img].rearrange("(n p) w -> p n w", p=128),
        )
        Kf = kt[:]
        Pf = pt[:]  # f32r (pre-rounded on host; bits are valid fp32 too)
        Pf32 = Pf.bitcast(F32)
        Pv = Pf32.rearrange("p (n w) -> p n w", n=4)

        # ---- x-branch (free axis) on DVE ----
        t1x = t1x_pool.tile([128, 2048], F32)
        _stencil_free_axis(nc, t1x[:].rearrange("p (n w) -> p n w", n=4), Pv)
        t2x = t2x_pool.tile([128, 2048], F32)
        nc.vector.tensor_tensor(t2x[:], Kf, t1x[:], _MUL)
        a_t = a_pool.tile([128, 2048], F32R)  # DVE converts on write
        _stencil_free_axis(
            nc,
            a_t[:].rearrange("p (n w) -> p n w", n=4),
            t2x[:].rearrange("p (n w) -> p n w", n=4),
        )

        # ---- y-branch (partition axis) on PE ----
        t1y = p_t1y.tile([128, 2048], F32)  # 4 banks, one 512-col chunk per i
        for i in range(4):
            js = [j for j in (i - 1, i, i + 1) if 0 <= j <= 3]
            for k, j in enumerate(js):
                nc.tensor.matmul(
                    t1y[:, i * 512 : (i + 1) * 512],
                    gtb(j, i),
                    Pf[:, j * 512 : (j + 1) * 512],
                    start=(k == 0),
                    stop=(k == len(js) - 1),
                )
        t2y = t2y_pool.tile([128, 2048], F32R)  # DVE converts on write
        nc.vector.tensor_tensor(t2y[:], Kf, t1y[:], _MUL)

        # ---- F' accumulation + fused square/row-sum ----
        c0 = img * 6
        for i in range(4):
            js = [j for j in (i - 1, i, i + 1) if 0 <= j <= 3]
            fp = p_fp.tile([128, 512], F32)
            for k, j in enumerate(js):
                nc.tensor.matmul(
                    fp[:],
                    gtb(j, i),
                    t2y[:, j * 512 : (j + 1) * 512],
                    start=(k == 0),
                    stop=False,
                )
            nc.tensor.matmul(
                fp[:], id_sb[:], a_t[:, i * 512 : (i + 1) * 512], start=False, stop=True
            )
            sq = sq_pool.tile([128, 512], F32)
            if i == 0:
                nc.scalar.activation(
                    sq[:, 0:64], fp[:, 0:64], _SQ,
                    bias=fb_sb[:, 0:1], scale=0.25,
                    accum_out=acc_sb[:, c0 : c0 + 1],
                )
                nc.scalar.activation(
                    sq[:, 64:512], fp[:, 64:512], _SQ, scale=0.25,
                    accum_out=acc_sb[:, c0 + 1 : c0 + 2],
                )
            elif i == 3:
                nc.scalar.activation(
                    sq[:, 0:448], fp[:, 0:448], _SQ, scale=0.25,
                    accum_out=acc_sb[:, c0 + 4 : c0 + 5],
                )
                nc.scalar.activation(
                    sq[:, 448:512], fp[:, 448:512], _SQ,
                    bias=fb_sb[:, 1:2], scale=0.25,
                    accum_out=acc_sb[:, c0 + 5 : c0 + 6],
                )
            else:
                nc.scalar.activation(
                    sq[:], fp[:], _SQ, scale=0.25,
                    accum_out=acc_sb[:, c0 + 1 + i : c0 + 2 + i],
                )

        # ---- MSE tiles (2 per image) ----
        for t in (2 * img, 2 * img + 1):
            mot = mo_pool.tile([128, 2048], F32)
            nc.sync.dma_start(mot[:].rearrange("p (n w) -> p n w", n=4), mo_r[t])
            tgt_ = tg_pool.tile([128, 2048], F32)
            nc.sync.dma_start(tgt_[:].rearrange("p (n w) -> p n w", n=4), tg_r[t])
            nc.vector.tensor_tensor(mot[:], mot[:], tgt_[:], _SUB)
            nc.scalar.activation(
                mot[:], mot[:], _SQ, accum_out=acc_sb[:, 24 + t : 25 + t]
            )

    # ---- final partition reduction: [128,32] -> [1,32] ----
    red = p_red.tile([1, 32], F32)
    nc.tensor.matmul(red[:], ones_sb[:], acc_sb[:])
    out_sb = consts.tile([1, 32], F32)
    nc.scalar.copy(out_sb[:], red[:])
    nc.sync.dma_start(out, out_sb[:])


_NC_CACHE = {}


def build_program():
    if "nc" in _NC_CACHE:
        return _NC_CACHE["nc"]
    nc = bacc.Bacc(
        "TRN2", target_bir_lowering=False, debug=False, num_devices=N_CORES
    )
    xk = nc.dram_tensor("xk", [BPC, H, W], F32, kind="ExternalInput").ap()
    xp = nc.dram_tensor("xp", [BPC, H, W], F32R, kind="ExternalInput").ap()
    mo = nc.dram_tensor("mo", [BPC, 2, H, W], F32, kind="ExternalInput").ap()
    tg = nc.dram_tensor("tg", [BPC, 2, H, W], F32, kind="ExternalInput").ap()
    gt = nc.dram_tensor("gt", [H, H], F32R, kind="ExternalInput").ap()
    ident = nc.dram_tensor("ident", [128, 128], F32R, kind="ExternalInput").ap()
    fbias = nc.dram_tensor("fbias", [128, 2], F32, kind="ExternalInput").ap()
    out = nc.dram_tensor("partials", [1, 32], F32, kind="ExternalOutput").ap()
    with tile.TileContext(nc) as tc, ExitStack() as ctx:
        _kernel_body(ctx, tc, xk, xp, mo, tg, gt, ident, fbias, out)
    nc.compile()
    _NC_CACHE["nc"] = nc
    return nc


def round_to_f32r(x):
    """Round fp32 values to the f32r grid (11 mantissa bits, RNE).

    f32r bit layout is fp32 with the low 12 mantissa bits zero, so the
    pre-rounded array is both a valid f32r payload and the exact fp32
    value the PE will use — keeping host emulation and DVE reads consistent.
    """
    u = np.ascontiguousarray(x, dtype=np.float32).view(np.uint32)
    lsb = (u >> np.uint32(12)) & np.uint32(1)
    r = (u + np.uint32(0x7FF) + lsb) & np.uint32(0xFFFFF000)
    return r.view(np.float32)


def make_in_maps(model_output, target, x0_hat):
    gt_np = np.ascontiguousarray(grad_matrix_2x(H).T)  # entries exact in f32r
    id_np = np.eye(128, dtype=np.float32)
    fb_np = np.zeros((128, 2), np.float32)
    fb_np[0:64, 0] = 10.0  # f source, rows 0:64 of block n=0 (cols 0:64)
    fb_np[64:128, 1] = -10.0  # f source, rows 448:512 of block n=3 (cols 448:512)
    x0_hat = np.asarray(x0_hat, dtype=np.float32)
    in_maps = []
    for c in range(N_CORES):
        sl = slice(c * BPC, (c + 1) * BPC)
        in_maps.append(
            {
                "xk": np.ascontiguousarray(x0_hat[sl, 0]),
                "xp": round_to_f32r(x0_hat[sl, 1]),
                "mo": np.ascontiguousarray(model_output[sl], dtype=np.float32),
                "tg": np.ascontiguousarray(target[sl], dtype=np.float32),
                "gt": gt_np,
                "ident": id_np,
                "fbias": fb_np,
            }
        )
    return in_maps


def combine_partials(partials_per_core, sigma_t):
    """partials: per core [1,32] f32 -> final scalar loss (host f64 math)."""
    total_mse = 0.0
    total_pde = 0.0
    for c in range(N_CORES):
        p = np.asarray(partials_per_core[c]).reshape(32).astype(np.float64)
        total_mse += p[24:32].sum()
        for img in range(BPC):
            s = p[img * 6 : (img + 1) * 6].sum()
            total_pde += 50.0 * float(sigma_t[c * BPC + img]) * s / float(H * W)
    loss = total_mse / float(B * 2 * H * W) + total_pde / float(B)
    return np.float32(loss)


def kernel(model_output, target, x0_hat, sigma_t):
    nc = build_program()
    in_maps = make_in_maps(model_output, target, x0_hat)
    res = bass_utils.run_bass_kernel_spmd(nc, in_maps, core_ids=list(range(N_CORES)))
    partials = [res.results[c]["partials"] for c in range(N_CORES)]
    return combine_partials(partials, np.asarray(sigma_t))


# revision 13
# speedup vs baseline: 1.7613x; 1.4389x over previous
"""Trainium2 Bass kernel for nn_DarcyLoss (data-parallel over batch on 8 cores).

loss = mean((model_output - target)^2)
     + mean_b( 0.5 * (sigma_t/0.01) * mean_hw(F_b^2) )
where F = dx(K * dx p) + dy(K * dy p) + f   (2nd-order finite differences,
K = x0_hat[:,0], p = x0_hat[:,1], f = Darcy source term).

Per-core plan (4 images each):
 - Work with the scaled stencil G' = 2*G (integer coefficients, exact in f32).
 - y-derivatives (partition axis): PE matmuls against constant G'^T blocks.
 - x-derivatives (free axis): DVE shifted-AP subtracts + 2-op edge fixups.
 - A (x-part) added into the F' PSUM accumulation via identity matmul.
 - F^2 = Square(0.25*F' + f) fused on ScalarE with accum_out row-sums.
 - MSE: DVE subtract (in-place), ScalarE Square with accum_out.
 - Partition reduction: single ones-matmul -> [1,32] partials -> host f64.
"""

import sys
from contextlib import ExitStack

import ml_dtypes
import numpy as np

sys.path.insert(0, "/opt/trn_rl_repo")

import concourse.bass as bass  # noqa: E402
import concourse.tile as tile  # noqa: E402
from concourse import bacc, mybir  # noqa: E402
from concourse import bass_utils  # noqa: E402

N_CORES = 8
B, H, W = 32, 512, 512
BPC = B // N_CORES  # images per core
F32 = mybir.dt.float32
BF16 = mybir.dt.bfloat16

_SUB = mybir.AluOpType.subtract
_ADD = mybir.AluOpType.add
_MUL = mybir.AluOpType.mult
_SQ = mybir.ActivationFunctionType.Square


def grad_matrix_2x(n: int) -> np.ndarray:
    """G' = 2 * (torch.gradient, spacing=1, edge_order=2) as a dense matrix."""
    G = np.zeros((n, n), np.float32)
    for h in range(1, n - 1):
        G[h, h + 1] = 1.0
        G[h, h - 1] = -1.0
    G[0, 0], G[0, 1], G[0, 2] = -3.0, 4.0, -1.0
    G[n - 1, n - 1], G[n - 1, n - 2], G[n - 1, n - 3] = 3.0, -4.0, 1.0
    return G


def _stencil_free_axis(nc, dst, src):
    """Apply G' along the last (free) axis: dst/src are [128, 4, 512] APs."""
    dre = dst
    # interior: dst[..., j] = src[..., j+1] - src[..., j-1]
    nc.vector.tensor_tensor(dst[:, :, 1:511], src[:, :, 2:512], src[:, :, 0:510], _SUB)
    # left edge: -3*s0 + 4*s1 - s2  (two fused scalar_tensor_tensor ops)
    nc.vector.scalar_tensor_tensor(
        dst[:, :, 0:1], src[:, :, 0:1], 3.0, src[:, :, 2:3], _MUL, _ADD
    )
    nc.vector.scalar_tensor_tensor(
        dst[:, :, 0:1], src[:, :, 1:2], 4.0, dre[:, :, 0:1], _MUL, _SUB
    )
    # right edge: 3*s511 - 4*s510 + s509
    nc.vector.scalar_tensor_tensor(
        dst[:, :, 511:512], src[:, :, 510:511], 4.0, src[:, :, 509:510], _MUL, _SUB
    )
    nc.vector.scalar_tensor_tensor(
        dst[:, :, 511:512], src[:, :, 511:512], 3.0, dre[:, :, 511:512], _MUL, _SUB
    )


def _kernel_body(ctx, tc, xk, xp, mo, tg, gt, ident, fbias, out):
    nc = tc.nc

    consts = ctx.enter_context(tc.tile_pool(name="consts", bufs=1))
    gt_sb = consts.tile([128, 2048], BF16)
    nc.sync.dma_start(
        gt_sb[:].rearrange("p (jb c) -> p jb c", jb=4),
        gt.rearrange("(jb p) c -> p jb c", p=128),
    )
    id_sb = consts.tile([128, 128], BF16)
    nc.sync.dma_start(id_sb[:], ident)
    fb_sb = consts.tile([128, 2], F32)
    nc.sync.dma_start(fb_sb[:], fbias)
    ones_sb = consts.tile([128, 1], F32)
    nc.vector.memset(ones_sb[:], 1.0)
    acc_sb = consts.tile([128, 32], F32)

    xk_pool = ctx.enter_context(tc.tile_pool(name="xk", bufs=2))
    xp_pool = ctx.enter_context(tc.tile_pool(name="xp", bufs=2))
    t1x_pool = ctx.enter_context(tc.tile_pool(name="t1x", bufs=2))
    t2x_pool = ctx.enter_context(tc.tile_pool(name="t2x", bufs=2))
    a_pool = ctx.enter_context(tc.tile_pool(name="abuf", bufs=2))
    t2y_pool = ctx.enter_context(tc.tile_pool(name="t2y", bufs=2))
    sq_pool = ctx.enter_context(tc.tile_pool(name="sq", bufs=3))
    mo_pool = ctx.enter_context(tc.tile_pool(name="mo", bufs=2))
    tg_pool = ctx.enter_context(tc.tile_pool(name="tg", bufs=2))
    p_t1y = ctx.enter_context(tc.tile_pool(name="pt1y", bufs=1, space="PSUM"))
    p_fp = ctx.enter_context(tc.tile_pool(name="pfp", bufs=2, space="PSUM"))
    p_red = ctx.enter_context(tc.tile_pool(name="pred", bufs=1, space="PSUM"))

    mo_r = mo.rearrange("b c (n p) w -> (b c) p n w", p=128)
    tg_r = tg.rearrange("b c (n p) w -> (b c) p n w", p=128)

    def gtb(j, i):
        """lhsT block: G'^T[j*128:(j+1)*128, i*128:(i+1)*128]."""
        return gt_sb[:, j * 512 + i * 128 : j * 512 + (i + 1) * 128]

    for img in range(BPC):
        kt = xk_pool.tile([128, 2048], BF16)
        nc.sync.dma_start(
            kt[:].rearrange("p (n w) -> p n w", n=4),
            xk[img].rearrange("(n p) w -> p n w", p=128),
        )
        pt = xp_pool.tile([128, 2048], BF16)
        nc.sync.dma_start(
            pt[:].rearrange("p (n w) -> p n w", n=4),
            xp[img].rearrange("(n p) w -> p n w", p=128),
        )
        Kf = kt[:]
        Pf = pt[:]
        Pv = Pf.rearrange("p (n w) -> p n w", n=4)

        # ---- x-branch (free axis) on DVE ----
        t1x = t1x_pool.tile([128, 2048], BF16)
        _stencil_free_axis(nc, t1x[:].rearrange("p (n w) -> p n w", n=4), Pv)
        t2x = t2x_pool.tile([128, 2048], BF16)
        nc.vector.tensor_tensor(t2x[:], Kf, t1x[:], _MUL)
        a_t = a_pool.tile([128, 2048], BF16)
        _stencil_free_axis(
            nc,
            a_t[:].rearrange("p (n w) -> p n w", n=4),
            t2x[:].rearrange("p (n w) -> p n w", n=4),
        )

        # ---- y-branch (partition axis) on PE ----
        t1y = p_t1y.tile([128, 2048], F32)  # 4 banks, one 512-col chunk per i
        for i in range(4):
            js = [j for j in (i - 1, i, i + 1) if 0 <= j <= 3]
            for k, j in enumerate(js):
                nc.tensor.matmul(
                    t1y[:, i * 512 : (i + 1) * 512],
                    gtb(j, i),
                    Pf[:, j * 512 : (j + 1) * 512],
                    start=(k == 0),
                    stop=(k == len(js) - 1),
                )
        t2y = t2y_pool.tile([128, 2048], BF16)
        nc.vector.tensor_tensor(t2y[:], Kf, t1y[:], _MUL)

        # ---- F' accumulation + fused square/row-sum ----
        c0 = img * 6
        for i in range(4):
            js = [j for j in (i - 1, i, i + 1) if 0 <= j <= 3]
            fp = p_fp.tile([128, 512], F32)
            for k, j in enumerate(js):
                nc.tensor.matmul(
                    fp[:],
                    gtb(j, i),
                    t2y[:, j * 512 : (j + 1) * 512],
                    start=(k == 0),
                    stop=False,
                )
            nc.tensor.matmul(
                fp[:], id_sb[:], a_t[:, i * 512 : (i + 1) * 512], start=False, stop=True
            )
            sq = sq_pool.tile([128, 512], F32)
            if i == 0:
                nc.scalar.activation(
                    sq[:, 0:64], fp[:, 0:64], _SQ,
                    bias=fb_sb[:, 0:1], scale=0.25,
                    accum_out=acc_sb[:, c0 : c0 + 1],
                )
                nc.scalar.activation(
                    sq[:, 64:512], fp[:, 64:512], _SQ, scale=0.25,
                    accum_out=acc_sb[:, c0 + 1 : c0 + 2],
                )
            elif i == 3:
                nc.scalar.activation(
                    sq[:, 0:448], fp[:, 0:448], _SQ, scale=0.25,
                    accum_out=acc_sb[:, c0 + 4 : c0 + 5],
                )
                nc.scalar.activation(
                    sq[:, 448:512], fp[:, 448:512], _SQ,
                    bias=fb_sb[:, 1:2], scale=0.25,
                    accum_out=acc_sb[:, c0 + 5 : c0 + 6],
                )
            else:
                nc.scalar.activation(
                    sq[:], fp[:], _SQ, scale=0.25,
                    accum_out=acc_sb[:, c0 + 1 + i : c0 + 2 + i],
                )

        # ---- MSE tiles (2 per image) ----
        for t in (2 * img, 2 * img + 1):
            mot = mo_pool.tile([128, 2048], BF16)
            nc.sync.dma_start(mot[:].rearrange("p (n w) -> p n w", n=4), mo_r[t])
            tgt_ = tg_pool.tile([128, 2048], BF16)
            nc.sync.dma_start(tgt_[:].rearrange("p (n w) -> p n w", n=4), tg_r[t])
            nc.vector.tensor_tensor(mot[:], mot[:], tgt_[:], _SUB)
            nc.scalar.activation(
                mot[:], mot[:], _SQ, accum_out=acc_sb[:, 24 + t : 25 + t]
            )

    # ---- final partition reduction: [128,32] -> [1,32] ----
    red = p_red.tile([1, 32], F32)
    nc.tensor.matmul(red[:], ones_sb[:], acc_sb[:])
    out_sb = consts.tile([1, 32], F32)
    nc.scalar.copy(out_sb[:], red[:])
    nc.sync.dma_start(out, out_sb[:])


_NC_CACHE = {}


def build_program():
    if "nc" in _NC_CACHE:
        return _NC_CACHE["nc"]
    nc = bacc.Bacc(
        "TRN2", target_bir_lowering=False, debug=False, num_devices=N_CORES
    )
    xk = nc.dram_tensor("xk", [BPC, H, W], BF16, kind="ExternalInput").ap()
    xp = nc.dram_tensor("xp", [BPC, H, W], BF16, kind="ExternalInput").ap()
    mo = nc.dram_tensor("mo", [BPC, 2, H, W], BF16, kind="ExternalInput").ap()
    tg = nc.dram_tensor("tg", [BPC, 2, H, W], BF16, kind="ExternalInput").ap()
    gt = nc.dram_tensor("gt", [H, H], BF16, kind="ExternalInput").ap()
    ident = nc.dram_tensor("ident", [128, 128], BF16, kind="ExternalInput").ap()
    fbias = nc.dram_tensor("fbias", [128, 2], F32, kind="ExternalInput").ap()
    out = nc.dram_tensor("partials", [1, 32], F32, kind="ExternalOutput").ap()
    with tile.TileContext(nc) as tc, ExitStack() as ctx:
        _kernel_body(ctx, tc, xk, xp, mo, tg, gt, ident, fbias, out)
    nc.compile()
    _NC_CACHE["nc"] = nc
    return nc


def make_in_maps(model_output, target, x0_hat):
    gt_np = np.ascontiguousarray(grad_matrix_2x(H).T).astype(ml_dtypes.bfloat16)
    id_np = np.eye(128, dtype=np.float32).astype(ml_dtypes.bfloat16)
    fb_np = np.zeros((128, 2), np.float32)
    fb_np[0:64, 0] = 10.0  # f source, rows 0:64 of block n=0 (cols 0:64)
    fb_np[64:128, 1] = -10.0  # f source, rows 448:512 of block n=3 (cols 448:512)
    bf16 = ml_dtypes.bfloat16
    x0_hat = np.asarray(x0_hat, dtype=np.float32)
    mo_b = np.ascontiguousarray(model_output, dtype=np.float32).astype(bf16)
    tg_b = np.ascontiguousarray(target, dtype=np.float32).astype(bf16)
    xk_b = np.ascontiguousarray(x0_hat[:, 0]).astype(bf16)
    xp_b = np.ascontiguousarray(x0_hat[:, 1]).astype(bf16)
    in_maps = []
    for c in range(N_CORES):
        sl = slice(c * BPC, (c + 1) * BPC)
        in_maps.append(
            {
                "xk": xk_b[sl],
                "xp": xp_b[sl],
                "mo": mo_b[sl],
                "tg": tg_b[sl],
                "gt": gt_np,
                "ident": id_np,
                "fbias": fb_np,
            }
        )
    return in_maps


def combine_partials(partials_per_core, sigma_t):
    """partials: per core [1,32] f32 -> final scalar loss (host f64 math)."""
    total_mse = 0.0
    total_pde = 0.0
    for c in range(N_CORES):
        p = np.asarray(partials_per_core[c]).reshape(32).astype(np.float64)
        total_mse += p[24:32].sum()
        for img in range(BPC):
            s = p[img * 6 : (img + 1) * 6].sum()
            total_pde += 50.0 * float(sigma_t[c * BPC + img]) * s / float(H * W)
    loss = total_mse / float(B * 2 * H * W) + total_pde / float(B)
    return np.float32(loss)


def kernel(model_output, target, x0_hat, sigma_t):
    nc = build_program()
    in_maps = make_in_maps(model_output, target, x0_hat)
    res = bass_utils.run_bass_kernel_spmd(nc, in_maps, core_ids=list(range(N_CORES)))
    partials = [res.results[c]["partials"] for c in range(N_CORES)]
    return combine_partials(partials, np.asarray(sigma_t))
